# revision 10
# baseline (speedup 1.0000x reference)
"""Trainium2 Bass kernel for the DNF (semi-symbolic dense MLP) problem.

Reference computation (per layer, x:(b,in), W:(out,in)):
    abs_w   = |x[:,i,None] * W.T[None,i,o]|          # (b, in, out)
    max_abs = max_i abs_w ; sum_abs = sum_i abs_w
    out     = x @ W.T + delta * (+/-)(max_abs - sum_abs)
Layer 1 (conjunction, +): tanh applied; layer 2 (disjunction, -).

max_i |x_i w_oi| is estimated with the ratio-of-power-sums
    0.1*max ~= sum_i 0.1|x w|^33 / sum_i (x w)^32
computed as two extra bf16 matmuls over element-wise powered operands
(POW32/POW33: fused squaring-chain custom DVE ops).

Structure: layer 1 is split into two o-halves A (cols 0:256) and B
(256:512) queued back-to-back on the PE; half A's epilogue chain
(recip/mult/sub -> tanh -> transpose -> conj powers) runs on the other
engines while half B's matmuls stream, and layer 2's contraction
chunks oc0/oc1 run between the A and B transposes.  This keeps the PE
continuously busy and lands layer 2 inside the HAM full-speed window.
Weight power tensors (fc1, gc1) come precomputed from the host (bf16
DMA); x-side powers and |w1| are derived on-device on idle engines.
"""

import numpy as np
import ml_dtypes

BATCH = 1024
NPRED = 512   # layer-1 contraction (in)
NCONJ = 512   # layer-1 out / layer-2 contraction
NOUT = 128    # layer-2 out
NCORES = 8
BSH = BATCH // NCORES  # 128 batch rows per core
KC1 = NPRED // 128
KC2 = NCONJ // 128

W1SC = 3.0   # global scale for layer-1 power tensors
W2SC = 2.0   # global scale for layer-2 power tensors
DELTA = 0.1

BF16 = ml_dtypes.bfloat16

_CACHE = {}


def _register_pow_ops():
    """POW32S: (s0*x)^32 ; POW33S: (s0*x)^33 - fused DVE squaring chains."""
    if "pow_ops" in _CACHE:
        return _CACHE["pow_ops"]
    import concourse.dve_ops as DO
    from concourse.dve_spec import Spec, Src0, C0, sq, lower
    from concourse.dve_spec import _has_src1 as has_src1
    from concourse.dve_uop import DveOpSpec

    def make(name, spec):
        for prev in DO.OPS:
            if prev.name == name:  # already registered (re-import)
                return prev
        opcode = DO._CUSTOM_DVE_ROW_BASE + len(DO.OPS)
        assert opcode < 0x20
        op = DO.DveOp(name, spec, subdim=False, uops_sha={})
        DO.OPS.append(op)
        DO._SUB_OPCODE_FOR_NAME[name] = opcode
        DO.CUSTOM_DVE_SPECS[name] = spec
        for ver in ("v3",):
            compiled = DveOpSpec(
                name=name, opcode=opcode,
                uops=lower(spec, ver=ver), rd1_en=has_src1(spec),
            )
            op.uops_sha[ver] = compiled.sha(ver)
        return op

    t = Src0 * C0
    pow32 = make(
        "POW32S_ANT",
        Spec(body=sq(sq(sq(sq(sq(t))))),
             reference=lambda in0, in1, c0, c1, c2: (
                 (np.float32(c0) * in0.astype(np.float32)) ** 32)),
    )
    t2 = Src0 * C0
    pow33 = make(
        "POW33S_ANT",
        Spec(body=sq(sq(sq(sq(sq(t2))))) * t2,
             reference=lambda in0, in1, c0, c1, c2: (
                 (np.float32(c0) * in0.astype(np.float32)) ** 33)),
    )
    _CACHE["pow_ops"] = (pow32, pow33)
    return pow32, pow33


def _build_nc():
    import concourse.mybir as mybir
    import concourse.tile as tile
    from concourse import bacc

    fp32 = mybir.dt.float32
    bf16 = mybir.dt.bfloat16
    AF = mybir.ActivationFunctionType
    ALU = mybir.AluOpType

    POW32, POW33 = _register_pow_ops()

    nc = bacc.Bacc("TRN2", debug=False)

    def dram_in(name, shape):
        return nc.dram_tensor(name, shape, bf16, kind="ExternalInput").ap()

    xt_d = dram_in("xt", (128, KC1, BSH))
    w1t_d = dram_in("w1t", (128, KC1, NCONJ))
    fc1_d = dram_in("fc1", (128, KC1, NCONJ))   # (3 W1.T)^32
    gc1_d = dram_in("gc1", (128, KC1, NCONJ))   # 3^32 |W1.T|^33
    w2_d = dram_in("w2all", (128, 2, KC2, NOUT))  # [w2t, w2a]
    id_d = dram_in("ident", (128, 128))
    out_d = nc.dram_tensor("out", (BSH, NOUT), fp32, kind="ExternalOutput").ap()

    # ga = 0.1|x|^33 (pairs with gc1 = 3^32|w|^33 against sp1's 3^32)
    GA_S = float(DELTA ** (1.0 / 33) / DELTA)
    # gc2 = 2^32|w|^33 so ga2 = 0.1|c|^33 (= fa2*ca) gives ratio 0.1*max
    GC2_S = float(W2SC ** (32.0 / 33))

    def flat(t):
        return t.rearrange("p a b -> p (a b)")

    HA = slice(0, 256)      # layer-1 o-half A
    HB = slice(256, 512)    # layer-1 o-half B

    with tile.TileContext(nc) as tc:
        with (
            tc.tile_pool(name="sb", bufs=1) as sb,
            tc.tile_pool(name="ptr", bufs=1, space="PSUM") as ptr,
            tc.tile_pool(name="pmm", bufs=5, space="PSUM") as pmm,
        ):
            # ---------------- SBUF tiles ----------------
            xt = sb.tile([128, KC1, BSH], bf16, tag="xt")
            xa = sb.tile([128, KC1, BSH], bf16, tag="xa")
            fa = sb.tile([128, KC1, BSH], bf16, tag="fa")
            ga = sb.tile([128, KC1, BSH], bf16, tag="ga")
            w1t = sb.tile([128, KC1, NCONJ], bf16, tag="w1t")
            fc1 = sb.tile([128, KC1, NCONJ], bf16, tag="fc1")
            w1a = sb.tile([128, KC1, NCONJ], bf16, tag="w1a")
            gc1 = sb.tile([128, KC1, NCONJ], bf16, tag="gc1")
            w2 = sb.tile([128, 2, KC2, NOUT], bf16, tag="w2")
            fc2 = sb.tile([128, KC2, NOUT], bf16, tag="fc2")
            gc2 = sb.tile([128, KC2, NOUT], bf16, tag="gc2")
            ident = sb.tile([128, 128], bf16, tag="ident")
            dmy = sb.tile([128, 128], bf16, tag="dmy")
            dmy2 = sb.tile([128, NCONJ], bf16, tag="dmy2")

            # ---------------- PE warm-up (HAM ramp) --------------------
            nc.vector.memset(dmy, 1.0)
            nc.vector.memset(dmy2, 1.0)
            wp = pmm.tile([128, NCONJ], fp32, tag="psum")
            for _ in range(4):
                nc.tensor.matmul(wp, dmy, dmy2, start=True, stop=True)

            # ---------------- input DMAs (critical first) --------------
            for ic in range(KC1):
                nc.sync.dma_start(out=w1t[:, ic, :], in_=w1t_d[:, ic, :])
            for ic in range(KC1):
                nc.sync.dma_start(out=gc1[:, ic, :], in_=gc1_d[:, ic, :])
            nc.sync.dma_start(out=ident, in_=id_d)
            nc.gpsimd.dma_start(out=xt, in_=xt_d)
            nc.gpsimd.dma_start(out=fc1[:, 0, :], in_=fc1_d[:, 0, :])
            nc.gpsimd.dma_start(out=fc1[:, 1, :], in_=fc1_d[:, 1, :])
            nc.gpsimd.dma_start(out=w2, in_=w2_d)
            nc.scalar.dma_start(out=fc1[:, 2, :], in_=fc1_d[:, 2, :])
            nc.scalar.dma_start(out=fc1[:, 3, :], in_=fc1_d[:, 3, :])

            # ---------------- on-device operand prep -------------------
            # scalar: |w1| per chunk + 0.1|x|
            for ic in range(KC1):
                nc.scalar.activation(w1a[:, ic, :], w1t[:, ic, :], AF.Abs)
                if ic == 0:
                    nc.scalar.activation(flat(xa), flat(xt), AF.Abs,
                                         scale=DELTA)
            # vector: x^32, 0.1|x|^33, layer-2 weight powers
            nc.vector._custom_dve(POW32, out=flat(fa), in0=flat(xt), s0=1.0)
            nc.vector._custom_dve(POW33, out=flat(ga), in0=flat(xa), s0=GA_S)
            nc.vector._custom_dve(POW32, out=flat(fc2), in0=flat(w2[:, 0]),
                                  s0=W2SC)
            nc.vector._custom_dve(POW33, out=flat(gc2), in0=flat(w2[:, 1]),
                                  s0=GC2_S)

            # ---------------- layer-1 matmuls: half A then half B ------
            mm1 = pmm.tile([128, NCONJ], fp32, tag="psum")
            s1 = pmm.tile([128, NCONJ], fp32, tag="psum")
            sp1 = pmm.tile([128, NCONJ], fp32, tag="psum")
            sq1 = pmm.tile([128, NCONJ], fp32, tag="psum")
            GROUPS1 = (
                (mm1, xt, w1t),
                (s1, xa, w1a),
                (sp1, fa, fc1),
                (sq1, ga, gc1),
            )
            for half in (HA, HB):
                for psum, lhs, rhs in GROUPS1:
                    for ic in range(KC1):
                        nc.tensor.matmul(
                            psum[:, half], lhs[:, ic, :], rhs[:, ic, half],
                            start=(ic == 0), stop=(ic == KC1 - 1),
                        )

            # ---------------- layer-1 epilogue (halved chains) ---------
            mm1n = sb.tile([128, NCONJ], fp32, tag="mm1n")
            z1 = sb.tile([128, NCONJ], fp32, tag="z1")
            rp1 = sb.tile([128, NCONJ], fp32, tag="rp1")
            tq1 = sb.tile([128, NCONJ], fp32, tag="tq1")
            v2 = sb.tile([128, NCONJ], fp32, tag="v2")
            conj = sb.tile([128, NCONJ], bf16, tag="conj")
            cT_ps = ptr.tile([128, KC2, 128], bf16, tag="cT_ps")   # (o, b)
            cT = sb.tile([128, KC2, 128], bf16, tag="cT")
            ca = sb.tile([128, KC2, 128], bf16, tag="ca")
            fa2 = sb.tile([128, KC2, 128], bf16, tag="fa2")
            ga2 = sb.tile([128, KC2, 128], bf16, tag="ga2")

            for h, half in ((0, HA), (1, HB)):
                # scalar: -mm1 copy ; vector: z/recip/mult/sub ; then tanh
                nc.scalar.activation(mm1n[:, half], mm1[:, half], AF.Copy,
                                     scale=-1.0)
                nc.vector.tensor_tensor(out=z1[:, half], in0=s1[:, half],
                                        in1=mm1n[:, half], op=ALU.add)
                nc.vector.reciprocal_approx_fast(out=rp1[:, half],
                                                 in_=sp1[:, half])
                nc.vector.tensor_tensor(out=tq1[:, half], in0=sq1[:, half],
                                        in1=rp1[:, half], op=ALU.mult)
                nc.vector.tensor_tensor(out=v2[:, half], in0=z1[:, half],
                                        in1=tq1[:, half], op=ALU.subtract)
                nc.scalar.activation(conj[:, half], v2[:, half], AF.Tanh,
                                     scale=-1.0)

            # ---------------- transposes + conj prep + layer 2 ---------
            # PE order: T01, L2 oc0/oc1 (all four groups), T23, L2 oc2/oc3
            sp2 = pmm.tile([128, NOUT], fp32, tag="psum")
            s2 = pmm.tile([128, NOUT], fp32, tag="psum")
            sq2 = pmm.tile([128, NOUT], fp32, tag="psum")
            mm2 = pmm.tile([128, NOUT], fp32, tag="psum")
            GROUPS2 = (
                (sp2, fa2, fc2[:, :, :]),
                (s2, ca, w2[:, 1]),
                (sq2, ga2, gc2[:, :, :]),
                (mm2, cT, w2[:, 0]),
            )

            for h in range(2):
                sl = slice(2 * h, 2 * h + 2)
                for oc in (2 * h, 2 * h + 1):
                    nc.tensor.transpose(
                        cT_ps[:, oc, :],
                        conj[:, oc * 128:(oc + 1) * 128],
                        ident,
                    )
                # preps for this half on vector/scalar/pool
                nc.vector._custom_dve(POW32, out=flat(fa2[:, sl, :]),
                                      in0=flat(cT_ps[:, sl, :]), s0=1.0)
                nc.scalar.activation(flat(ca[:, sl, :]), flat(cT_ps[:, sl, :]),
                                     AF.Abs, scale=DELTA)
                nc.scalar.activation(flat(cT[:, sl, :]), flat(cT_ps[:, sl, :]),
                                     AF.Copy)
                nc.gpsimd.tensor_tensor(out=flat(ga2[:, sl, :]),
                                        in0=flat(fa2[:, sl, :]),
                                        in1=flat(ca[:, sl, :]), op=ALU.mult)
                # layer-2 contraction chunks for this half
                for psum, lhs, rhs in GROUPS2:
                    for oc in (2 * h, 2 * h + 1):
                        nc.tensor.matmul(
                            psum, lhs[:, oc, :], rhs[:, oc, :],
                            start=(oc == 0), stop=(oc == KC2 - 1),
                        )

            # ---------------- layer-2 epilogue ----------------
            rp2 = sb.tile([128, NOUT], fp32, tag="rp2")
            nc.vector.reciprocal_approx_fast(out=rp2, in_=sp2)
            tq2 = sb.tile([128, NOUT], fp32, tag="tq2")
            nc.vector.tensor_tensor(out=tq2, in0=sq2, in1=rp2, op=ALU.mult)
            u1 = sb.tile([128, NOUT], fp32, tag="u1")
            nc.vector.tensor_tensor(out=u1, in0=s2, in1=tq2, op=ALU.subtract)
            res = sb.tile([128, NOUT], fp32, tag="res")
            nc.vector.tensor_tensor(out=res, in0=mm2, in1=u1, op=ALU.add)
            nc.sync.dma_start(out=out_d[:, 0:64], in_=res[:, 0:64])
            nc.gpsimd.dma_start(out=out_d[:, 64:128], in_=res[:, 64:128])

    nc.compile()
    return nc


def _get_nc():
    if "nc" not in _CACHE:
        _CACHE["nc"] = _build_nc()
    return _CACHE["nc"]


def _perm(a, kc):
    """(128*kc, n) -> (128, kc, n) with partition = index % 128."""
    n = a.shape[1]
    return np.ascontiguousarray(
        a.reshape(kc, 128, n).transpose(1, 0, 2))


def _prep_inputs(x, W_conj, W_disj):
    """Host-side (free) prep: shard x, weight transposes + powers, bf16."""
    x = np.asarray(x, dtype=np.float32)
    W1 = np.asarray(W_conj, dtype=np.float64)
    W2 = np.asarray(W_disj, dtype=np.float32)

    w1T = W1.T
    w1t = _perm(w1T.astype(np.float32), KC1).astype(BF16)
    fc1 = _perm((W1SC * np.abs(w1T)) ** 32, KC1).astype(BF16)
    gc1 = _perm(W1SC ** 32 * np.abs(w1T) ** 33, KC1).astype(BF16)
    w2t = _perm(W2.T, KC2).astype(BF16)
    w2a = _perm(np.abs(W2.T), KC2).astype(BF16)
    w2all = np.ascontiguousarray(np.stack([w2t, w2a], axis=1))
    ident = np.eye(128, dtype=BF16)

    in_maps = []
    for c in range(NCORES):
        xs = x[c * BSH:(c + 1) * BSH].T        # (in, b)
        in_maps.append({
            "xt": _perm(xs, KC1).astype(BF16),
            "w1t": w1t,
            "fc1": fc1,
            "gc1": gc1,
            "w2all": w2all,
            "ident": ident,
        })
    return in_maps


def kernel(x: np.ndarray, W_conj: np.ndarray, W_disj: np.ndarray) -> np.ndarray:
    from concourse.bass_utils import run_bass_kernel_spmd

    nc = _get_nc()
    in_maps = _prep_inputs(x, W_conj, W_disj)
    res = run_bass_kernel_spmd(nc, in_maps, core_ids=list(range(NCORES)))
    return np.concatenate([r["out"] for r in res.results], axis=0)


# revision 14
# speedup vs baseline: 1.0843x; 1.0843x over previous
"""Trainium2 Bass kernel for the DNF (semi-symbolic dense MLP) problem.

Reference computation (per layer, x:(b,in), W:(out,in)):
    abs_w   = |x[:,i,None] * W.T[None,i,o]|          # (b, in, out)
    max_abs = max_i abs_w ; sum_abs = sum_i abs_w
    out     = x @ W.T + delta * (+/-)(max_abs - sum_abs)
Layer 1 (conjunction, +): tanh applied; layer 2 (disjunction, -).

max_i |x_i w_oi| is estimated with the ratio-of-power-sums
    0.1*max ~= sum_i 0.1|x w|^33 / sum_i (x w)^32
computed as two extra bf16 matmuls over element-wise powered operands
(POW32/POW33: fused squaring-chain custom DVE ops).  Odd powers are
either POW33 or even-power * abs (pool multiply).

All matmuls are bf16 single-pass.  Only x.T, W1.T, [W2.T | |W2.T|]
and an identity are DMA'd (~0.93MB, critical chunks first); all other
operands are derived on-device on whichever engine has slack.

The layer-1 -> layer-2 junction is halved: the epilogue chain, tanh,
transposes, conj powers and layer-2 contraction chunks are split into
o-halves A/B with SEPARATE tiles per half (the tile framework tracks
dependencies at tile granularity), so half B's chain runs while half
A's layer-2 chunks stream on the PE.
"""

import numpy as np
import ml_dtypes

BATCH = 1024
NPRED = 512   # layer-1 contraction (in)
NCONJ = 512   # layer-1 out / layer-2 contraction
NOUT = 128    # layer-2 out
NCORES = 8
BSH = BATCH // NCORES  # 128 batch rows per core
KC1 = NPRED // 128
KC2 = NCONJ // 128

W1SC = 3.0   # global scale for layer-1 power tensors
W2SC = 2.0   # global scale for layer-2 power tensors
DELTA = 0.1

BF16 = ml_dtypes.bfloat16

_CACHE = {}


def _register_pow_ops():
    """POW32S: (s0*x)^32 ; POW33S: (s0*x)^33 - fused DVE squaring chains."""
    if "pow_ops" in _CACHE:
        return _CACHE["pow_ops"]
    import concourse.dve_ops as DO
    from concourse.dve_spec import Spec, Src0, C0, sq, lower
    from concourse.dve_spec import _has_src1 as has_src1
    from concourse.dve_uop import DveOpSpec

    def make(name, spec):
        for prev in DO.OPS:
            if prev.name == name:  # already registered (re-import)
                return prev
        opcode = DO._CUSTOM_DVE_ROW_BASE + len(DO.OPS)
        assert opcode < 0x20
        op = DO.DveOp(name, spec, subdim=False, uops_sha={})
        DO.OPS.append(op)
        DO._SUB_OPCODE_FOR_NAME[name] = opcode
        DO.CUSTOM_DVE_SPECS[name] = spec
        for ver in ("v3",):
            compiled = DveOpSpec(
                name=name, opcode=opcode,
                uops=lower(spec, ver=ver), rd1_en=has_src1(spec),
            )
            op.uops_sha[ver] = compiled.sha(ver)
        return op

    t = Src0 * C0
    pow32 = make(
        "POW32S_ANT",
        Spec(body=sq(sq(sq(sq(sq(t))))),
             reference=lambda in0, in1, c0, c1, c2: (
                 (np.float32(c0) * in0.astype(np.float32)) ** 32)),
    )
    t2 = Src0 * C0
    pow33 = make(
        "POW33S_ANT",
        Spec(body=sq(sq(sq(sq(sq(t2))))) * t2,
             reference=lambda in0, in1, c0, c1, c2: (
                 (np.float32(c0) * in0.astype(np.float32)) ** 33)),
    )
    _CACHE["pow_ops"] = (pow32, pow33)
    return pow32, pow33


def _build_nc():
    import concourse.mybir as mybir
    import concourse.tile as tile
    from concourse import bacc

    fp32 = mybir.dt.float32
    bf16 = mybir.dt.bfloat16
    AF = mybir.ActivationFunctionType
    ALU = mybir.AluOpType

    POW32, POW33 = _register_pow_ops()

    nc = bacc.Bacc("TRN2", debug=False)

    xt_d = nc.dram_tensor("xt", (128, KC1, BSH), bf16,
                          kind="ExternalInput").ap()
    w1t_d = nc.dram_tensor("w1t", (128, KC1, NCONJ), bf16,
                           kind="ExternalInput").ap()
    w2_d = nc.dram_tensor("w2all", (128, 2, KC2, NOUT), bf16,
                          kind="ExternalInput").ap()   # [w2t, w2a]
    id_d = nc.dram_tensor("ident", (128, 128), bf16,
                          kind="ExternalInput").ap()
    out_d = nc.dram_tensor("out", (BSH, NOUT), fp32, kind="ExternalOutput").ap()

    # gc1 uniform 3^32|w|^33 (vector POW33 s0=3^(32/33); pool fc1*|w1a|),
    # paired with ga = 0.1|x|^33 against sp1's 3^32 -> ratio = 0.1*max.
    # Layer 2: gc2 = 2^32|w|^33, ga2 = fa2*ca = 0.1|c|^33.
    GA_S = float(DELTA ** (1.0 / 33) / DELTA)
    GC1_S = float(W1SC ** (32.0 / 33))
    GC2_S = float(W2SC ** (32.0 / 33))

    def flat(t):
        return t.rearrange("p a b -> p (a b)")

    HALVES = (slice(0, 256), slice(256, 512))

    with tile.TileContext(nc) as tc:
        with (
            tc.tile_pool(name="sb", bufs=1) as sb,
            tc.tile_pool(name="ptr", bufs=1, space="PSUM") as ptr,
            tc.tile_pool(name="pmm", bufs=4, space="PSUM") as pmm,
        ):
            # ---------------- SBUF tiles ----------------
            xt = sb.tile([128, KC1, BSH], bf16, tag="xt")
            xa = sb.tile([128, KC1, BSH], bf16, tag="xa")
            fa = sb.tile([128, KC1, BSH], bf16, tag="fa")
            ga = sb.tile([128, KC1, BSH], bf16, tag="ga")
            w1t = sb.tile([128, KC1, NCONJ], bf16, tag="w1t")
            fc1 = sb.tile([128, KC1, NCONJ], bf16, tag="fc1")
            w1a = sb.tile([128, KC1, NCONJ], bf16, tag="w1a")
            gc1 = sb.tile([128, KC1, NCONJ], bf16, tag="gc1")
            w2 = sb.tile([128, 2, KC2, NOUT], bf16, tag="w2")
            fc2 = sb.tile([128, KC2, NOUT], bf16, tag="fc2")
            gc2 = sb.tile([128, KC2, NOUT], bf16, tag="gc2")
            ident = sb.tile([128, 128], bf16, tag="ident")
            dmy = sb.tile([128, 128], bf16, tag="dmy")
            dmy2 = sb.tile([128, NCONJ], bf16, tag="dmy2")

            # ---------------- PE warm-up (HAM ramp) --------------------
            nc.vector.memset(dmy, 1.0)
            nc.vector.memset(dmy2, 1.0)
            wp = ptr.tile([128, NCONJ], fp32, tag="wp")
            for _ in range(4):
                nc.tensor.matmul(wp, dmy, dmy2, start=True, stop=True)

            # ---------------- input DMAs (critical first) --------------
            for ic in range(KC1):
                nc.sync.dma_start(out=w1t[:, ic, :], in_=w1t_d[:, ic, :])
            nc.sync.dma_start(out=w2, in_=w2_d)
            nc.sync.dma_start(out=ident, in_=id_d)
            nc.gpsimd.dma_start(out=xt, in_=xt_d)

            # ---------------- on-device operand prep -------------------
            for ic in range(KC1):
                nc.scalar.activation(w1a[:, ic, :], w1t[:, ic, :], AF.Abs)
                if ic == 0:
                    nc.scalar.activation(flat(xa), flat(xt), AF.Abs,
                                         scale=DELTA)
            nc.vector._custom_dve(POW32, out=flat(fa), in0=flat(xt), s0=1.0)
            for ic in range(KC1):
                nc.vector._custom_dve(POW32, out=fc1[:, ic, :],
                                      in0=w1t[:, ic, :], s0=W1SC)
            nc.vector._custom_dve(POW33, out=flat(ga), in0=flat(xa), s0=GA_S)
            for ic in range(2):
                nc.vector._custom_dve(POW33, out=gc1[:, ic, :],
                                      in0=w1a[:, ic, :], s0=GC1_S)
            for ic in range(2, KC1):
                nc.gpsimd.tensor_tensor(out=gc1[:, ic, :], in0=fc1[:, ic, :],
                                        in1=w1a[:, ic, :], op=ALU.mult)

            # ---------------- layer-1 matmuls (full width) -------------
            mm1 = pmm.tile([128, NCONJ], fp32, tag="psum")
            s1 = pmm.tile([128, NCONJ], fp32, tag="psum")
            sp1 = pmm.tile([128, NCONJ], fp32, tag="psum")
            sq1 = pmm.tile([128, NCONJ], fp32, tag="psum")
            for psum, lhs, rhs in (
                (mm1, xt, w1t),
                (s1, xa, w1a),
                (sp1, fa, fc1),
                (sq1, ga, gc1),
            ):
                for ic in range(KC1):
                    nc.tensor.matmul(
                        psum, lhs[:, ic, :], rhs[:, ic, :],
                        start=(ic == 0), stop=(ic == KC1 - 1),
                    )

            # ---------------- layer-1 epilogue (separate half tiles) ---
            mm1n = sb.tile([128, NCONJ], fp32, tag="mm1n")
            nc.scalar.activation(mm1n, mm1, AF.Copy, scale=-1.0)
            nc.vector._custom_dve(POW32, out=flat(fc2), in0=flat(w2[:, 0]),
                                  s0=W2SC)
            nc.vector._custom_dve(POW33, out=flat(gc2), in0=flat(w2[:, 1]),
                                  s0=GC2_S)
            z1 = sb.tile([128, NCONJ], fp32, tag="z1")
            nc.vector.tensor_tensor(out=z1, in0=s1, in1=mm1n, op=ALU.add)

            rp1 = [sb.tile([128, 256], fp32, name=f"rp1{h}", tag=f"rp1{h}")
                   for h in range(2)]
            tq1 = [sb.tile([128, 256], fp32, name=f"tq1{h}", tag=f"tq1{h}")
                   for h in range(2)]
            v2 = [sb.tile([128, 256], fp32, name=f"v2{h}", tag=f"v2{h}")
                  for h in range(2)]
            conj = [sb.tile([128, 256], bf16, name=f"conj{h}", tag=f"conj{h}")
                    for h in range(2)]
            for h, half in enumerate(HALVES):
                nc.vector.reciprocal_approx_fast(out=rp1[h], in_=sp1[:, half])
            for h, half in enumerate(HALVES):
                nc.vector.tensor_tensor(out=tq1[h], in0=sq1[:, half],
                                        in1=rp1[h], op=ALU.mult)
                nc.vector.tensor_tensor(out=v2[h], in0=z1[:, half],
                                        in1=tq1[h], op=ALU.subtract)
                nc.scalar.activation(conj[h], v2[h], AF.Tanh, scale=-1.0)

            # ---------------- transposes (all before layer 2) ----------
            cT_ps = [ptr.tile([128, 2, 128], bf16, name=f"cT_ps{h}",
                              tag=f"cT_ps{h}") for h in range(2)]
            for h in range(2):
                for j in range(2):
                    nc.tensor.transpose(
                        cT_ps[h][:, j, :],
                        conj[h][:, j * 128:(j + 1) * 128],
                        ident,
                    )

            # ---------------- conj prep (per half) ----------------
            cT = [sb.tile([128, 2, 128], bf16, name=f"cT{h}", tag=f"cT{h}")
                  for h in range(2)]
            ca = [sb.tile([128, 2, 128], bf16, name=f"ca{h}", tag=f"ca{h}")
                  for h in range(2)]
            fa2 = [sb.tile([128, 2, 128], bf16, name=f"fa2{h}", tag=f"fa2{h}")
                   for h in range(2)]
            ga2 = [sb.tile([128, 2, 128], bf16, name=f"ga2{h}", tag=f"ga2{h}")
                   for h in range(2)]
            for h in range(2):
                nc.vector._custom_dve(POW32, out=flat(fa2[h]),
                                      in0=flat(cT_ps[h]), s0=1.0)
                nc.vector.tensor_copy(flat(cT[h]), flat(cT_ps[h]))
                nc.scalar.activation(flat(ca[h]), flat(cT_ps[h]), AF.Abs,
                                     scale=DELTA)
                nc.gpsimd.tensor_tensor(out=flat(ga2[h]), in0=flat(fa2[h]),
                                        in1=flat(ca[h]), op=ALU.mult)

            # ---------------- layer-2 matmuls (oc pairs) ----------------
            sp2 = pmm.tile([128, NOUT], fp32, tag="psum")
            s2 = pmm.tile([128, NOUT], fp32, tag="psum")
            sq2 = pmm.tile([128, NOUT], fp32, tag="psum")
            mm2 = pmm.tile([128, NOUT], fp32, tag="psum")
            for h in range(2):
                order = ((sp2, fa2[h], fc2), (s2, ca[h], w2[:, 1]),
                         (mm2, cT[h], w2[:, 0]), (sq2, ga2[h], gc2))
                if h == 1:   # end on mm2 so res follows the last matmul
                    order = ((sp2, fa2[h], fc2), (s2, ca[h], w2[:, 1]),
                             (sq2, ga2[h], gc2), (mm2, cT[h], w2[:, 0]))
                for psum, lhs, rhs in order:
                    for j in range(2):
                        oc = 2 * h + j
                        nc.tensor.matmul(
                            psum, lhs[:, j, :], rhs[:, oc, :],
                            start=(oc == 0), stop=(oc == KC2 - 1),
                        )

            # ---------------- layer-2 epilogue ----------------
            rp2 = sb.tile([128, NOUT], fp32, tag="rp2")
            nc.vector.reciprocal_approx_fast(out=rp2, in_=sp2)
            tq2 = sb.tile([128, NOUT], fp32, tag="tq2")
            nc.vector.tensor_tensor(out=tq2, in0=sq2, in1=rp2, op=ALU.mult)
            u1 = sb.tile([128, NOUT], fp32, tag="u1")
            nc.vector.tensor_tensor(out=u1, in0=s2, in1=tq2, op=ALU.subtract)
            res = sb.tile([128, NOUT], fp32, tag="res")
            nc.vector.tensor_tensor(out=res, in0=mm2, in1=u1, op=ALU.add)
            nc.sync.dma_start(out=out_d[:, 0:64], in_=res[:, 0:64])
            nc.gpsimd.dma_start(out=out_d[:, 64:128], in_=res[:, 64:128])

    nc.compile()
    return nc


def _get_nc():
    if "nc" not in _CACHE:
        _CACHE["nc"] = _build_nc()
    return _CACHE["nc"]


def _perm(a, kc):
    """(128*kc, n) -> (128, kc, n) with partition = index % 128."""
    n = a.shape[1]
    return np.ascontiguousarray(
        a.reshape(kc, 128, n).transpose(1, 0, 2))


def _prep_inputs(x, W_conj, W_disj):
    """Host-side (free) prep: shard x, transpose weights, all bf16."""
    x = np.asarray(x, dtype=np.float32)
    W1 = np.asarray(W_conj, dtype=np.float32)
    W2 = np.asarray(W_disj, dtype=np.float32)

    w1t = _perm(W1.T, KC1).astype(BF16)
    w2t = _perm(W2.T, KC2).astype(BF16)
    w2a = _perm(np.abs(W2.T), KC2).astype(BF16)
    w2all = np.ascontiguousarray(np.stack([w2t, w2a], axis=1))
    ident = np.eye(128, dtype=BF16)

    in_maps = []
    for c in range(NCORES):
        xs = x[c * BSH:(c + 1) * BSH].T        # (in, b)
        in_maps.append({
            "xt": _perm(xs, KC1).astype(BF16),
            "w1t": w1t,
            "w2all": w2all,
            "ident": ident,
        })
    return in_maps


def kernel(x: np.ndarray, W_conj: np.ndarray, W_disj: np.ndarray) -> np.ndarray:
    from concourse.bass_utils import run_bass_kernel_spmd

    nc = _get_nc()
    in_maps = _prep_inputs(x, W_conj, W_disj)
    res = run_bass_kernel_spmd(nc, in_maps, core_ids=list(range(NCORES)))
    return np.concatenate([r["out"] for r in res.results], axis=0)


# revision 16
# speedup vs baseline: 1.2765x; 1.1772x over previous
"""Trainium2 Bass kernel for the DNF (semi-symbolic dense MLP) problem.

Reference computation (per layer, x:(b,in), W:(out,in)):
    abs_w   = |x[:,i,None] * W.T[None,i,o]|          # (b, in, out)
    max_abs = max_i abs_w ; sum_abs = sum_i abs_w
    out     = x @ W.T + delta * (+/-)(max_abs - sum_abs)
Layer 1 (conjunction, +): tanh applied; layer 2 (disjunction, -).

max_i |x_i w_oi| is estimated with a single-sided p-norm:
    max ~= (sum_i (s*x*w)^32)^(1/32) / s
The 32nd root (and the delta/s scale) is ONE vector tensor_scalar op
via the bitcast fast-root: bitcast(bitcast_int(sp) >> 5 + K) with
K = 127*2^23*31/32 + log2(delta/s)*2^23.  End-to-end numpy emulation
gives rel err ~1.5e-3 (tolerance 2e-2).

Per layer only three bf16 matmul groups remain (x@W.T, 0.1|x|@|W|.T,
x^32@(sW)^32).  Even powers are POW32 (fused squaring-chain custom DVE
op, sign-free).  Only x.T, W1.T, [W2.T||W2.T|] and an identity are
DMA'd (~0.93MB, critical chunks first); |x|, |w1| on scalar, powers on
vector.  The layer-1 -> layer-2 junction (fast-root, subtract, tanh,
transpose, conj powers, layer-2 contraction chunks) is split into
o-halves with separate tiles per half so the halves pipeline.
"""

import math

import numpy as np
import ml_dtypes

BATCH = 1024
NPRED = 512   # layer-1 contraction (in)
NCONJ = 512   # layer-1 out / layer-2 contraction
NOUT = 128    # layer-2 out
NCORES = 8
BSH = BATCH // NCORES  # 128 batch rows per core
KC1 = NPRED // 128
KC2 = NCONJ // 128

W1SC = 3.0   # global scale for layer-1 power tensors
W2SC = 2.0   # global scale for layer-2 power tensors
DELTA = 0.1

BF16 = ml_dtypes.bfloat16

_CACHE = {}


def _fastroot_k(c):
    """Magic constant: bitcast(i>>5 + K) ~= c * x^(1/32)."""
    return int(round(127 * (1 << 23) * 31 / 32 + math.log2(c) * (1 << 23)))


def _register_pow32():
    """POW32S: (s0*x)^32 as one fused squaring-chain DVE op."""
    if "pow32" in _CACHE:
        return _CACHE["pow32"]
    import concourse.dve_ops as DO
    from concourse.dve_spec import Spec, Src0, C0, sq, lower
    from concourse.dve_spec import _has_src1 as has_src1
    from concourse.dve_uop import DveOpSpec

    name = "POW32S_ANT"
    op = None
    for prev in DO.OPS:
        if prev.name == name:  # already registered (re-import)
            op = prev
    if op is None:
        opcode = DO._CUSTOM_DVE_ROW_BASE + len(DO.OPS)
        assert opcode < 0x20
        t = Src0 * C0
        spec = Spec(
            body=sq(sq(sq(sq(sq(t))))),
            reference=lambda in0, in1, c0, c1, c2: (
                (np.float32(c0) * in0.astype(np.float32)) ** 32),
        )
        op = DO.DveOp(name, spec, subdim=False, uops_sha={})
        DO.OPS.append(op)
        DO._SUB_OPCODE_FOR_NAME[name] = opcode
        DO.CUSTOM_DVE_SPECS[name] = spec
        for ver in ("v3",):
            compiled = DveOpSpec(
                name=name, opcode=opcode,
                uops=lower(spec, ver=ver), rd1_en=has_src1(spec),
            )
            op.uops_sha[ver] = compiled.sha(ver)
    _CACHE["pow32"] = op
    return op


def _build_nc():
    import concourse.mybir as mybir
    import concourse.tile as tile
    from concourse import bacc

    fp32 = mybir.dt.float32
    bf16 = mybir.dt.bfloat16
    i32 = mybir.dt.int32
    AF = mybir.ActivationFunctionType
    ALU = mybir.AluOpType

    POW32 = _register_pow32()

    nc = bacc.Bacc("TRN2", debug=False)

    xt_d = nc.dram_tensor("xt", (128, KC1, BSH), bf16,
                          kind="ExternalInput").ap()
    w1t_d = nc.dram_tensor("w1t", (128, KC1, NCONJ), bf16,
                           kind="ExternalInput").ap()
    w2_d = nc.dram_tensor("w2all", (128, 2, KC2, NOUT), bf16,
                          kind="ExternalInput").ap()   # [w2t, w2a]
    id_d = nc.dram_tensor("ident", (128, 128), bf16,
                          kind="ExternalInput").ap()
    out_d = nc.dram_tensor("out", (BSH, NOUT), fp32, kind="ExternalOutput").ap()

    K1 = _fastroot_k(DELTA / W1SC)   # tq1 = 0.1 * max1 from sp1
    K2 = _fastroot_k(DELTA / W2SC)   # tq2 = 0.1 * max2 from sp2

    def flat(t):
        return t.rearrange("p a b -> p (a b)")

    HALVES = (slice(0, 256), slice(256, 512))

    with tile.TileContext(nc) as tc:
        with (
            tc.tile_pool(name="sb", bufs=1) as sb,
            tc.tile_pool(name="ptr", bufs=1, space="PSUM") as ptr,
            tc.tile_pool(name="pmm", bufs=3, space="PSUM") as pmm,
        ):
            # ---------------- SBUF tiles ----------------
            xt = sb.tile([128, KC1, BSH], bf16, tag="xt")
            xa = sb.tile([128, KC1, BSH], bf16, tag="xa")
            fa = sb.tile([128, KC1, BSH], bf16, tag="fa")
            w1t = sb.tile([128, KC1, NCONJ], bf16, tag="w1t")
            fc1 = sb.tile([128, KC1, NCONJ], bf16, tag="fc1")
            w1a = sb.tile([128, KC1, NCONJ], bf16, tag="w1a")
            w2 = sb.tile([128, 2, KC2, NOUT], bf16, tag="w2")
            fc2 = sb.tile([128, KC2, NOUT], bf16, tag="fc2")
            ident = sb.tile([128, 128], bf16, tag="ident")
            dmy = sb.tile([128, 128], bf16, tag="dmy")
            dmy2 = sb.tile([128, NCONJ], bf16, tag="dmy2")

            # ---------------- PE warm-up (HAM ramp) --------------------
            nc.vector.memset(dmy, 1.0)
            nc.vector.memset(dmy2, 1.0)
            wp = ptr.tile([128, NCONJ], fp32, tag="wp")
            for _ in range(4):
                nc.tensor.matmul(wp, dmy, dmy2, start=True, stop=True)

            # ---------------- input DMAs (critical first) --------------
            for ic in range(KC1):
                nc.sync.dma_start(out=w1t[:, ic, :], in_=w1t_d[:, ic, :])
            nc.sync.dma_start(out=w2, in_=w2_d)
            nc.sync.dma_start(out=ident, in_=id_d)
            nc.gpsimd.dma_start(out=xt, in_=xt_d)

            # ---------------- on-device operand prep -------------------
            for ic in range(KC1):
                nc.scalar.activation(w1a[:, ic, :], w1t[:, ic, :], AF.Abs)
                if ic == 0:
                    nc.scalar.activation(flat(xa), flat(xt), AF.Abs,
                                         scale=DELTA)
            nc.vector._custom_dve(POW32, out=flat(fa), in0=flat(xt), s0=1.0)
            for ic in range(KC1):
                nc.vector._custom_dve(POW32, out=fc1[:, ic, :],
                                      in0=w1t[:, ic, :], s0=W1SC)
            nc.vector._custom_dve(POW32, out=flat(fc2), in0=flat(w2[:, 0]),
                                  s0=W2SC)

            # ---------------- layer-1 matmuls (full width) -------------
            mm1 = pmm.tile([128, NCONJ], fp32, tag="psum")
            s1 = pmm.tile([128, NCONJ], fp32, tag="psum")
            sp1 = pmm.tile([128, NCONJ], fp32, tag="psum")
            for psum, lhs, rhs in (
                (mm1, xt, w1t),
                (s1, xa, w1a),
                (sp1, fa, fc1),
            ):
                for ic in range(KC1):
                    nc.tensor.matmul(
                        psum, lhs[:, ic, :], rhs[:, ic, :],
                        start=(ic == 0), stop=(ic == KC1 - 1),
                    )

            # ---------------- layer-1 epilogue (halved chains) ---------
            mm1n = sb.tile([128, NCONJ], fp32, tag="mm1n")
            nc.scalar.activation(mm1n, mm1, AF.Copy, scale=-1.0)
            z1 = sb.tile([128, NCONJ], fp32, tag="z1")
            nc.vector.tensor_tensor(out=z1, in0=s1, in1=mm1n, op=ALU.add)

            tq1 = [sb.tile([128, 256], fp32, name=f"tq1{h}", tag=f"tq1{h}")
                   for h in range(2)]
            v2 = [sb.tile([128, 256], fp32, name=f"v2{h}", tag=f"v2{h}")
                  for h in range(2)]
            conj = [sb.tile([128, 256], bf16, name=f"conj{h}", tag=f"conj{h}")
                    for h in range(2)]
            for h, half in enumerate(HALVES):
                # tq1 = 0.1*max ~= bitcast(int(sp1)>>5 + K1)
                nc.vector.tensor_scalar(
                    tq1[h].bitcast(i32), sp1[:, half].bitcast(i32),
                    5, None, ALU.logical_shift_right)
                nc.vector.tensor_scalar(
                    tq1[h].bitcast(i32), tq1[h].bitcast(i32),
                    K1, None, ALU.add)
                nc.vector.tensor_tensor(out=v2[h], in0=z1[:, half],
                                        in1=tq1[h], op=ALU.subtract)
                nc.scalar.activation(conj[h], v2[h], AF.Tanh, scale=-1.0)

            # ---------------- transposes + conj prep + layer 2 ---------
            cT_ps = [ptr.tile([128, 2, 128], bf16, name=f"cT_ps{h}",
                              tag=f"cT_ps{h}") for h in range(2)]
            cT = [sb.tile([128, 2, 128], bf16, name=f"cT{h}", tag=f"cT{h}")
                  for h in range(2)]
            ca = [sb.tile([128, 2, 128], bf16, name=f"ca{h}", tag=f"ca{h}")
                  for h in range(2)]
            fa2 = [sb.tile([128, 2, 128], bf16, name=f"fa2{h}", tag=f"fa2{h}")
                   for h in range(2)]
            sp2 = pmm.tile([128, NOUT], fp32, tag="psum")
            s2 = pmm.tile([128, NOUT], fp32, tag="psum")
            mm2 = pmm.tile([128, NOUT], fp32, tag="psum")
            for h in range(2):
                for j in range(2):
                    nc.tensor.transpose(
                        cT_ps[h][:, j, :],
                        conj[h][:, j * 128:(j + 1) * 128],
                        ident,
                    )
                nc.vector._custom_dve(POW32, out=flat(fa2[h]),
                                      in0=flat(cT_ps[h]), s0=1.0)
                nc.scalar.activation(flat(ca[h]), flat(cT_ps[h]), AF.Abs,
                                     scale=DELTA)
                nc.vector.tensor_copy(flat(cT[h]), flat(cT_ps[h]))
                for psum, lhs, rhs in (
                    (sp2, fa2[h], fc2),
                    (s2, ca[h], w2[:, 1]),
                    (mm2, cT[h], w2[:, 0]),
                ):
                    for j in range(2):
                        oc = 2 * h + j
                        nc.tensor.matmul(
                            psum, lhs[:, j, :], rhs[:, oc, :],
                            start=(oc == 0), stop=(oc == KC2 - 1),
                        )

            # ---------------- layer-2 epilogue ----------------
            tq2 = sb.tile([128, NOUT], fp32, tag="tq2")
            nc.vector.tensor_scalar(
                tq2.bitcast(i32), sp2.bitcast(i32),
                5, None, ALU.logical_shift_right)
            nc.vector.tensor_scalar(
                tq2.bitcast(i32), tq2.bitcast(i32),
                K2, None, ALU.add)
            u1 = sb.tile([128, NOUT], fp32, tag="u1")
            nc.vector.tensor_tensor(out=u1, in0=s2, in1=tq2, op=ALU.subtract)
            res = sb.tile([128, NOUT], fp32, tag="res")
            nc.vector.tensor_tensor(out=res, in0=mm2, in1=u1, op=ALU.add)
            nc.sync.dma_start(out=out_d[:, 0:64], in_=res[:, 0:64])
            nc.gpsimd.dma_start(out=out_d[:, 64:128], in_=res[:, 64:128])

    nc.compile()
    return nc


def _get_nc():
    if "nc" not in _CACHE:
        _CACHE["nc"] = _build_nc()
    return _CACHE["nc"]


def _perm(a, kc):
    """(128*kc, n) -> (128, kc, n) with partition = index % 128."""
    n = a.shape[1]
    return np.ascontiguousarray(
        a.reshape(kc, 128, n).transpose(1, 0, 2))


def _prep_inputs(x, W_conj, W_disj):
    """Host-side (free) prep: shard x, transpose weights, all bf16."""
    x = np.asarray(x, dtype=np.float32)
    W1 = np.asarray(W_conj, dtype=np.float32)
    W2 = np.asarray(W_disj, dtype=np.float32)

    w1t = _perm(W1.T, KC1).astype(BF16)
    w2t = _perm(W2.T, KC2).astype(BF16)
    w2a = _perm(np.abs(W2.T), KC2).astype(BF16)
    w2all = np.ascontiguousarray(np.stack([w2t, w2a], axis=1))
    ident = np.eye(128, dtype=BF16)

    in_maps = []
    for c in range(NCORES):
        xs = x[c * BSH:(c + 1) * BSH].T        # (in, b)
        in_maps.append({
            "xt": _perm(xs, KC1).astype(BF16),
            "w1t": w1t,
            "w2all": w2all,
            "ident": ident,
        })
    return in_maps


def kernel(x: np.ndarray, W_conj: np.ndarray, W_disj: np.ndarray) -> np.ndarray:
    from concourse.bass_utils import run_bass_kernel_spmd

    nc = _get_nc()
    in_maps = _prep_inputs(x, W_conj, W_disj)
    res = run_bass_kernel_spmd(nc, in_maps, core_ids=list(range(NCORES)))
    return np.concatenate([r["out"] for r in res.results], axis=0)
